# revision 18
# baseline (speedup 1.0000x reference)
"""Modulated 3x3 conv (StyleGAN2-style, groups=B) on 8 trn2 NeuronCores.

Sharding: data-parallel over (batch B=4) x (image half H/2), 8 shards.
Each core computes a full 64->64 channel 3x3 conv over a 256-row half of
one sample's 512x512 image, via Winograd F(2,3) along the W axis.

Why Winograd: the direct 6-matmul/row-pair schedule is PE-streaming-bound
at 768 N=512 matmuls/body, and the sustained HAM power throttle caps the
tensor engine at k=13/16 duty, so the only lever is fewer PE cycles.
F(2,3) along W replaces the 3 kw-shifted matmuls with 4 phase matmuls at
HALF the free dim, cutting streamed columns 1.5x: 512 N=512 matmuls/body.

Scheme per "quad" (2 output row pairs = 4 output rows):
  phases p=0..3 of the input rows (V0=Xe[j]-Xe[j+1], V1=Xo[j]+Xe[j+1],
  V2=Xe[j+1]-Xo[j], V3=Xo[j]-Xo[j+1], from a host-prepared even/odd
  column split), and kw-transformed stationaries A_p/B_p (g'=G g along kw,
  kh packed in quadrants as in the direct kernel; style s on K, demod d on
  M). PSUM tile = 2 banks: bank1 accumulates m1 then +m0, bank2 m2 then
  +m3 (4 matmuls each, N=512 covering both row pairs). Mid-accumulation
  ACT drains m1,m2 to SBUF; the inverse transform is then just two DVE
  combines: out_even = bank1 + m2_sb, out_odd = m1_sb - bank2.

All pools persist across the unrolled bodies; the next body's weight prep
is double-buffered and emitted inside the current body (DMA+vector chain
early, PE reduce matmuls + stationary builds late) so the PE stream never
idles at body boundaries (idle would also re-trigger HAM). Input
transforms run on DVE+GpSimd a group ahead of the matmuls; output stores
issue from the ACT HWDGE queue so the Sync queue carries only x loads.
The host only slices/pads/casts/relayouts (even/odd split, output
interleave decode); s and d are computed on-device per body.
"""

import numpy as np

import concourse.bacc as bacc
import concourse.mybir as mybir
import concourse.tile as tile
from concourse.bass_utils import run_bass_kernel_spmd

B, CIN, COUT, L, H, W = 4, 64, 64, 512, 512, 512
N_CORES = 8
HALF = H // 2  # 256 output rows per core
R_IN = HALF + 2  # 258 input rows per core (1 halo/pad row each side)
NPAIR = HALF // 2  # 128 output row pairs per core
NQUAD = NPAIR // 2  # 64
G = 8  # row pairs per group
NOG = NPAIR // G  # 16 groups
ROWB = W + 2  # per-row SBUF cols: [Xe 257 | Xo 257]
EO = ROWB // 2  # 257
NJ = W // 2  # 256 output col-pairs per row
EPS = 1e-8
F32 = mybir.dt.float32
F16 = mybir.dt.float16
XBUFS = 4  # raw x tile ring
PHBUFS = 3  # phase tile ring
PW = (G + 1) * NJ  # phase block stride (9 pairs x 256)
PREP_EARLY = 2
PREP_LATE = 12

PA_COLS = 2 * L + 1  # 1025: affw dup | w bcast | affb dup
PB_COLS = COUT * 9 + 6 * 128  # 1344: wgt_t | six raw stationaries

_CACHE = {}


def _build_nc(reps=1, hw_iters=1):
    nc = bacc.Bacc("TRN2", target_bir_lowering=False, debug=False)
    # xs[s, ci, q, :]: padded row 2q+s, cols = [Xe(257) | Xo(257)]
    xs = nc.dram_tensor("xs", [2, CIN, R_IN // 2, ROWB], F16, kind="ExternalInput")
    prepA = nc.dram_tensor("prepA", [128, PA_COLS], F32, kind="ExternalInput")
    prepB = nc.dram_tensor("prepB", [128, PB_COLS], F32, kind="ExternalInput")
    # out[s, co, group, quad, eo, c]: c = pair-in-quad*256 + j; col = 2j+eo
    out = nc.dram_tensor("out", [2, COUT, NOG, 4, 2, W], F16, kind="ExternalOutput")

    with tile.TileContext(nc) as tc:
        if hw_iters > 1:
            with tc.For_i(0, hw_iters, 1):
                _emit_all(tc, xs, prepA, prepB, out, reps)
        else:
            _emit_all(tc, xs, prepA, prepB, out, reps)
    nc.compile()
    return nc


def _prep_early(tc, pools, prepA, prepB):
    """Prep phase A: packed DMAs + style/demod vector chains (Sync/DVE/ACT)."""
    nc = tc.nc
    wst = pools["wst"]
    pA = wst.tile([128, PA_COLS], F32, tag="pA")
    pB = wst.tile([128, PB_COLS], F32, tag="pB")
    nc.sync.dma_start(pA[:], prepA[:, :])
    nc.sync.dma_start(pB[:], prepB[:, :])
    # s2[p] = sum_l affw[p%64, l] * w[l] + affb[p%64] + 1  (gpsimd: SBUF-only
    # prep work stays off the saturated DVE)
    scr = wst.tile([128, L], F32, tag="scr")
    nc.gpsimd.tensor_mul(scr[:], pA[:, 0:L], pA[:, L : 2 * L])
    s2_raw = wst.tile([128, 1], F32, tag="s2_raw")
    nc.vector.reduce_sum(s2_raw[:], scr[:], axis=mybir.AxisListType.X)
    affb2p1 = wst.tile([128, 1], F32, tag="affb2p1")
    nc.scalar.activation(
        affb2p1[:], pA[:, 2 * L : 2 * L + 1],
        mybir.ActivationFunctionType.Identity, bias=1.0,
    )
    s2 = wst.tile([128, 1], F32, tag="s2")
    nc.scalar.activation(
        s2[:], s2_raw[:], mybir.ActivationFunctionType.Identity, bias=affb2p1[:]
    )
    s2h = wst.tile([128, 1], F32, tag="s2h")
    nc.gpsimd.tensor_scalar_mul(s2h[:], s2[:], 0.5)
    # qsum[ci, co] = sum_t (s2[ci] * wgt_t[ci, co*9+t])^2
    swT = wst.tile([64, COUT * 9], F32, tag="swT")
    nc.gpsimd.tensor_scalar_mul(swT[:], pB[0:64, 0 : COUT * 9], s2[0:64, :])
    nc.gpsimd.tensor_mul(swT[:], swT[:], swT[:])
    qsum = wst.tile([64, COUT], F32, tag="qsum")
    nc.vector.reduce_sum(
        qsum[:].rearrange("p (c u) -> p c u", u=1),
        swT[:].rearrange("p (c t) -> p c t", t=9),
        axis=mybir.AxisListType.X,
    )
    return {"pB": pB, "s2": s2, "s2h": s2h, "qsum": qsum}


def _prep_late(tc, pools, st, consts):
    """Prep phase B: demod broadcast (tiny PE matmuls, emitted late so the
    in-order PE queue never waits on them) + the 8 winograd stationaries."""
    nc = tc.nc
    wst = pools["wst"]
    ones64, onesrow = consts
    # dsq[co] = sum_ci qsum[ci, co]; d = rsqrt(dsq + eps), bcast to [128,128]
    ps_d = pools["pprep"].tile([1, COUT], F32, tag="pp")
    nc.tensor.matmul(ps_d[:], ones64[:], st["qsum"][:])
    drow = wst.tile([1, 128], F32, tag="drow")
    nc.vector.tensor_scalar_add(drow[:, 0:64], ps_d[:], EPS)
    nc.vector.tensor_scalar_add(drow[:, 64:128], ps_d[:], EPS)
    nc.vector.reciprocal(drow[:], drow[:])
    nc.scalar.activation(drow[:], drow[:], mybir.ActivationFunctionType.Sqrt)
    ps_dbc = pools["pprep"].tile([128, 128], F32, tag="pp")
    nc.tensor.matmul(ps_dbc[:], onesrow[:], drow[:])
    dbc = wst.tile([128, 128], F32, tag="dbc")
    nc.vector.tensor_copy(dbc[:], ps_dbc[:])
    # 8 stationaries: per set (A from kw tiles 0-2, B from 3-5):
    #   ST0=g0, ST1=(g0+g1+g2)/2, ST2=(g0-g1+g2)/2, ST3=g2; x s2 (K), x dbc (M)
    s2, s2h = st["s2"], st["s2h"]
    W0 = COUT * 9
    stat = []
    tsum = wst.tile([128, 128], F32, tag="tsum")
    tmid = wst.tile([128, 128], F32, tag="tmid")
    for base, nm in ((0, "A"), (3, "B")):
        g = [st["pB"][:, W0 + 128 * (base + k) : W0 + 128 * (base + k + 1)]
             for k in range(3)]
        nc.gpsimd.tensor_add(tsum[:], g[0], g[2])
        for p, (src, sc) in enumerate(
            ((g[0], s2), (None, s2h), (None, s2h), (g[2], s2))
        ):
            t = wst.tile([128, 128], F16, tag=f"{nm}{p}")
            if src is None:
                if p == 1:
                    nc.gpsimd.tensor_add(tmid[:], tsum[:], g[1])
                else:
                    nc.gpsimd.tensor_sub(tmid[:], tsum[:], g[1])
                src = tmid[:]
            tscaled = wst.tile([128, 128], F32, tag=f"{nm}{p}s")
            nc.gpsimd.tensor_scalar_mul(tscaled[:], src, sc[:])
            nc.gpsimd.tensor_mul(t[:], tscaled[:], dbc[:])
            stat.append(t)
    st["A"] = stat[0:4]
    st["B"] = stat[4:8]


def _emit_body(tc, pools, xs, out, st, prepA, prepB, consts, do_next, carry):
    """One conv body. `carry` holds the cross-body software-pipeline state:
    pending passB quad + last raw tile (the body halo is body-local, so the
    carry is only within-body; reset each body)."""
    nc = tc.nc
    xsv = xs.rearrange("s ci q c -> (s ci) q c")  # [128, 129, ROWB]
    outv = out.rearrange("s co g u e c -> (s co) (g u e c)")  # [128, 16*4*2*512]

    raws = {}  # raw tile index -> AP
    phases = {}  # phase group -> AP

    def load_raw(i):
        # 9-pair tiles overlapping by one pair: tile i = pairs 8i..8i+8, so a
        # transform group needs exactly one raw tile (no cross-tile halo op)
        if i in raws:
            return
        q0 = 8 * i
        t = pools["xg"].tile([128, (G + 1) * ROWB], F16, tag="xg")
        nc.sync.dma_start(
            t[:],
            xsv[:, q0 : q0 + G + 1, :].rearrange("p q c -> p (q c)"),
        )
        raws[i] = t

    def transform(g):
        """Build phase group g (pairs 8g .. 8g+8) from raw tile g (9 pairs)."""
        ph = pools["ph"].tile([128, 4 * PW], F16, tag="ph")
        big = raws[g][:].rearrange("p (b c) -> p b c", c=ROWB)
        # slices of a row: Xe[j]=0:256, Xe[j+1]=1:257, Xo[j]=257:513, Xo[j+1]=258:514
        ops = [
            (0, 0, 1, "sub"),      # V0 = Xe[j] - Xe[j+1]
            (1, 257, 1, "add"),    # V1 = Xo[j] + Xe[j+1]
            (2, 1, 257, "sub"),    # V2 = Xe[j+1] - Xo[j]
            (3, 257, 258, "sub"),  # V3 = Xo[j] - Xo[j+1]
        ]
        for p, a0, b0, op in ops:
            # DVE only: gpsimd tensor ops run at 0.42 efficiency and a full
            # phase op there (~4.6us) is reserved for the og_even combines
            f = nc.vector.tensor_sub if op == "sub" else nc.vector.tensor_add
            f(
                ph[:, p * PW : p * PW + PW].rearrange("p (b c) -> p b c", c=NJ),
                big[:, :, a0 : a0 + NJ],
                big[:, :, b0 : b0 + NJ],
            )
        phases[g] = ph

    def pass_a(u):
        """First 4 matmuls of quad u (m1 into bank1, m2 into bank2) + the
        mid-group ACT drains of m1 (f32, for og_odd on DVE) and m2 (f16,
        for og_even on GpSimd)."""
        g, qu = divmod(u, 4)
        ph = phases[g][:]
        ps = pools["ps"].tile([128, 1024], F32, tag="ps")
        m1sb = pools["msb"].tile([128, 512], F32, tag="m1sb")
        m2sb = pools["msb"].tile([128, 512], F16, tag="m2sb")
        qq = 2 * (qu % 4) * NJ  # rel col of pair 2u within the phase group
        for bank, p in ((0, 1), (1, 2)):
            bv = ps[:, bank * 512 : bank * 512 + 512]
            nc.tensor.matmul(bv, st["A"][p][:],
                             ph[:, p * PW + qq : p * PW + qq + 512],
                             start=True, stop=False)
            nc.tensor.matmul(bv, st["B"][p][:],
                             ph[:, p * PW + qq + NJ : p * PW + qq + NJ + 512],
                             start=False, stop=False)
        nc.scalar.activation(m1sb[:], ps[:, 0:512],
                             mybir.ActivationFunctionType.Copy)
        nc.scalar.activation(m2sb[:], ps[:, 512:1024],
                             mybir.ActivationFunctionType.Copy)
        return {"ps": ps, "m1sb": m1sb, "m2sb": m2sb, "u": u}

    def pass_b(qa, og):
        """Last 4 matmuls of quad u (+m0 into bank1, +m3 into bank2), ACT
        drain of bank1 (m0+m1, f16), then the inverse combines: og_even on
        GpSimd (all-f16 SBUF), og_odd on DVE (psum operand)."""
        u = qa["u"]
        g, qu = divmod(u, 4)
        ph = phases[g][:]
        ps = qa["ps"]
        qq = 2 * (qu % 4) * NJ
        for bank, p in ((0, 0), (1, 3)):
            bv = ps[:, bank * 512 : bank * 512 + 512]
            nc.tensor.matmul(bv, st["A"][p][:],
                             ph[:, p * PW + qq : p * PW + qq + 512],
                             start=False, stop=False)
            nc.tensor.matmul(bv, st["B"][p][:],
                             ph[:, p * PW + qq + NJ : p * PW + qq + NJ + 512],
                             start=False, stop=True)
        b1sb = pools["msb"].tile([128, 512], F16, tag="b1sb")
        nc.scalar.activation(b1sb[:], ps[:, 0:512],
                             mybir.ActivationFunctionType.Copy)
        o = og[:, qu * 1024 : qu * 1024 + 1024]
        nc.gpsimd.tensor_add(o[:, 0:512], b1sb[:], qa["m2sb"][:])
        nc.vector.tensor_sub(o[:, 512:1024], qa["m1sb"][:], ps[:, 512:1024])

    st_next = None
    load_raw(0)
    transform(0)
    load_raw(1)
    pend = None  # pass-A state of the previous quad (1-quad software lag)
    ogs = {}
    for g in range(NOG):
        load_raw(min(g + 2, NOG - 1))
        if g + 1 < NOG:
            transform(g + 1)
        if do_next and g == PREP_EARLY:
            st_next = _prep_early(tc, pools, prepA, prepB)
        if do_next and g == PREP_LATE:
            _prep_late(tc, pools, st_next, consts)
        ogs[g] = pools["og"].tile([128, 4 * 1024], F16, tag="og", name="og")
        for qu in range(4):
            u = 4 * g + qu
            qa = pass_a(u)
            if pend is not None:
                up = pend["u"]
                pass_b(pend, ogs[up // 4])
                if up % 4 == 3:
                    gp = up // 4
                    nc.scalar.dma_start(
                        outv[:, gp * 4096 : (gp + 1) * 4096], ogs[gp][:]
                    )
                    del ogs[gp]
            pend = qa
    pass_b(pend, ogs[NOG - 1])
    nc.scalar.dma_start(outv[:, (NOG - 1) * 4096 : NOG * 4096], ogs[NOG - 1][:])
    return st_next if st_next is not None else st


def _emit_all(tc, xs, prepA, prepB, out, reps):
    nc = tc.nc
    pools = {
        "const": tc.alloc_tile_pool(name="const", bufs=1),
        "wst": tc.alloc_tile_pool(name="wst", bufs=2),
        "pprep": tc.alloc_tile_pool(name="pprep", bufs=1, space="PSUM"),
        "xg": tc.alloc_tile_pool(name="xg", bufs=XBUFS),
        "ph": tc.alloc_tile_pool(name="ph", bufs=PHBUFS),
        "og": tc.alloc_tile_pool(name="og", bufs=3),
        "msb": tc.alloc_tile_pool(name="msb", bufs=3),
        "ps": tc.alloc_tile_pool(name="ps", bufs=3, space="PSUM"),
    }
    ones64 = pools["const"].tile([64, 1], F32, tag="ones64")
    nc.vector.memset(ones64[:], 1.0)
    onesrow = pools["const"].tile([1, 128], F32, tag="onesrow")
    nc.vector.memset(onesrow[:], 1.0)
    consts = (ones64, onesrow)

    st = _prep_early(tc, pools, prepA, prepB)
    _prep_late(tc, pools, st, consts)
    for r in range(reps):
        st = _emit_body(tc, pools, xs, out, st, prepA, prepB, consts,
                        do_next=(r < reps - 1), carry=None)
    for p in ["ps", "msb", "og", "ph", "xg", "pprep", "wst", "const"]:
        pools[p].release()


def _get_nc(reps=1, hw_iters=1):
    key = (reps, hw_iters)
    if key not in _CACHE:
        _CACHE[key] = _build_nc(reps, hw_iters)
    return _CACHE[key]


def _host_weight_layouts(weight):
    """prepB: wgt_t [ci, co*9] + six raw kw stationary tiles, packed."""
    lhsT6 = np.zeros((6, 128, 128), dtype=np.float32)
    wt = np.ascontiguousarray(weight.transpose(1, 0, 2, 3))  # [ci, co, kh, kw]
    for kw in range(3):
        a, b = lhsT6[kw], lhsT6[3 + kw]
        a[0:64, 0:64] = wt[:, :, 0, kw]
        a[64:128, 0:64] = wt[:, :, 1, kw]
        a[64:128, 64:128] = wt[:, :, 0, kw]
        b[0:64, 0:64] = wt[:, :, 2, kw]
        b[0:64, 64:128] = wt[:, :, 1, kw]
        b[64:128, 64:128] = wt[:, :, 2, kw]
    prepB = np.zeros((128, PB_COLS), dtype=np.float32)
    prepB[0:64, 0 : COUT * 9] = wt.reshape(CIN, COUT * 9)
    prepB[:, COUT * 9 :] = lhsT6.transpose(1, 0, 2).reshape(128, 6 * 128)
    return prepB


def _shard_inputs(x, w, weight, affine_w, affine_b):
    """Build the 8 per-core input maps (host-side slicing + halo padding +
    even/odd column split)."""
    prepB = _host_weight_layouts(np.asarray(weight, dtype=np.float32))
    affw = np.asarray(affine_w, dtype=np.float32)
    affb = np.asarray(affine_b, dtype=np.float32)
    w = np.asarray(w, dtype=np.float32)
    x = np.asarray(x, dtype=np.float32)
    prepA_base = np.zeros((128, PA_COLS), dtype=np.float32)
    prepA_base[0:64, 0:L] = affw
    prepA_base[64:128, 0:L] = affw
    prepA_base[0:64, 2 * L] = affb
    prepA_base[64:128, 2 * L] = affb
    in_maps = []
    for core in range(N_CORES):
        b, half = divmod(core, 2)
        h0 = half * HALF
        xsh = np.zeros((CIN, R_IN, ROWB), dtype=np.float32)
        lo, hi = h0 - 1, h0 + HALF + 1  # global rows [lo, hi)
        clo, chi = max(lo, 0), min(hi, H)
        xsh[:, clo - lo : chi - lo, 1 : 1 + W] = x[b, :, clo:chi, :]
        # even/odd column split per row: [Xe(257) | Xo(257)]
        xeo = np.concatenate([xsh[:, :, 0::2], xsh[:, :, 1::2]], axis=2)
        # parity-split layout: xs2[s, ci, q, :] = padded row 2q+s
        xs2 = np.ascontiguousarray(
            xeo.reshape(CIN, R_IN // 2, 2, ROWB).transpose(2, 0, 1, 3)
        ).astype(np.float16)
        prepA = prepA_base.copy()
        prepA[:, L : 2 * L] = w[b][None, :]
        in_maps.append({"xs": xs2, "prepA": prepA, "prepB": prepB})
    return in_maps


def _decode_core(o2):
    """[2,COUT,16,4,2,512] device layout -> [COUT, HALF, W] rows/cols."""
    # dims [s, co, U=(g,u), e, pp, j] -> row 4U+2pp+s, col 2j+e
    o6 = np.asarray(o2, dtype=np.float32).reshape(2, COUT, NQUAD, 2, 2, NJ)
    return o6.transpose(1, 2, 4, 0, 5, 3).reshape(COUT, HALF, W)


def kernel(x, w, weight, affine_w, affine_b):
    nc = _get_nc()
    in_maps = _shard_inputs(x, w, weight, affine_w, affine_b)
    res = run_bass_kernel_spmd(nc, in_maps, list(range(N_CORES)))
    full = np.empty((B, COUT, H, W), dtype=np.float32)
    for core in range(N_CORES):
        b, half = divmod(core, 2)
        full[b, :, half * HALF : (half + 1) * HALF, :] = _decode_core(
            res.results[core]["out"]
        )
    return full


# revision 19
# speedup vs baseline: 1.0635x; 1.0635x over previous
"""Modulated 3x3 conv (StyleGAN2-style, groups=B) on 8 trn2 NeuronCores.

Sharding: data-parallel over (batch B=4) x (image half H/2), 8 shards.
Each core computes a full 64->64 channel 3x3 conv over a 256-row half of
one sample's 512x512 image, via Winograd F(2,3) along the W axis.

Why Winograd: the direct 6-matmul/row-pair schedule is PE-streaming-bound
at 768 N=512 matmuls/body, and the sustained HAM power throttle caps the
tensor engine at k=13/16 duty, so the only lever is fewer PE cycles.
F(2,3) along W replaces the 3 kw-shifted matmuls with 4 phase matmuls at
HALF the free dim, cutting streamed columns 1.5x: 512 N=512 matmuls/body.

Scheme per "quad" (2 output row pairs = 4 output rows):
  phases p=0..3 of the input rows (V0=Xe[j]-Xe[j+1], V1=Xo[j]+Xe[j+1],
  V2=Xe[j+1]-Xo[j], V3=Xo[j]-Xo[j+1], from a host-prepared even/odd
  column split), and kw-transformed stationaries A_p/B_p (g'=G g along kw,
  kh packed in quadrants as in the direct kernel; style s on K, demod d on
  M). PSUM tile = 2 banks: bank1 accumulates m1 then +m0, bank2 m2 then
  +m3 (4 matmuls each, N=512 covering both row pairs). Mid-accumulation
  ACT drains m1,m2 to SBUF; the inverse transform is then just two DVE
  combines: out_even = bank1 + m2_sb, out_odd = m1_sb - bank2.

All pools persist across the unrolled bodies; the next body's weight prep
is double-buffered and emitted inside the current body (DMA+vector chain
early, PE reduce matmuls + stationary builds late) so the PE stream never
idles at body boundaries (idle would also re-trigger HAM). Input
transforms run on DVE+GpSimd a group ahead of the matmuls; output stores
issue from the ACT HWDGE queue so the Sync queue carries only x loads.
The host only slices/pads/casts/relayouts (even/odd split, output
interleave decode); s and d are computed on-device per body.
"""

import numpy as np

import concourse.bacc as bacc
import concourse.mybir as mybir
import concourse.tile as tile
from concourse.bass_utils import run_bass_kernel_spmd

B, CIN, COUT, L, H, W = 4, 64, 64, 512, 512, 512
N_CORES = 8
HALF = H // 2  # 256 output rows per core
R_IN = HALF + 2  # 258 input rows per core (1 halo/pad row each side)
NPAIR = HALF // 2  # 128 output row pairs per core
NQUAD = NPAIR // 2  # 64
G = 8  # row pairs per group
NOG = NPAIR // G  # 16 groups
ROWB = W + 2  # per-row SBUF cols: [Xe 257 | Xo 257]
EO = ROWB // 2  # 257
NJ = W // 2  # 256 output col-pairs per row
EPS = 1e-8
F32 = mybir.dt.float32
F16 = mybir.dt.float16
XBUFS = 4  # raw x tile ring
PHBUFS = 3  # phase tile ring
PW = (G + 1) * NJ  # phase block stride (9 pairs x 256)
PREP_EARLY = 2
PREP_LATE = 12

PA_COLS = 2 * L + 1  # 1025: affw dup | w bcast | affb dup
PB_COLS = COUT * 9 + 6 * 128  # 1344: wgt_t | six raw stationaries

_CACHE = {}


def _build_nc(reps=1, hw_iters=1):
    nc = bacc.Bacc("TRN2", target_bir_lowering=False, debug=False)
    # xs[s, ci, q, :]: padded row 2q+s, cols = [Xe(257) | Xo(257)]
    xs = nc.dram_tensor("xs", [2, CIN, R_IN // 2, ROWB], F16, kind="ExternalInput")
    prepA = nc.dram_tensor("prepA", [128, PA_COLS], F32, kind="ExternalInput")
    prepB = nc.dram_tensor("prepB", [128, PB_COLS], F32, kind="ExternalInput")
    # out[s, co, group, quad, eo, c]: c = pair-in-quad*256 + j; col = 2j+eo
    out = nc.dram_tensor("out", [2, COUT, NOG, 4, 2, W], F16, kind="ExternalOutput")

    with tile.TileContext(nc) as tc:
        if hw_iters > 1:
            with tc.For_i(0, hw_iters, 1):
                _emit_all(tc, xs, prepA, prepB, out, reps)
        else:
            _emit_all(tc, xs, prepA, prepB, out, reps)
    nc.compile()
    return nc


def _prep_early(tc, pools, prepA, prepB):
    """Prep phase A: packed DMAs + style/demod vector chains (Sync/DVE/ACT)."""
    nc = tc.nc
    wst = pools["wst"]
    pA = wst.tile([128, PA_COLS], F32, tag="pA")
    pB = wst.tile([128, PB_COLS], F32, tag="pB")
    nc.sync.dma_start(pA[:], prepA[:, :])
    nc.sync.dma_start(pB[:], prepB[:, :])
    # s2[p] = sum_l affw[p%64, l] * w[l] + affb[p%64] + 1  (gpsimd: SBUF-only
    # prep work stays off the saturated DVE)
    scr = wst.tile([128, L], F32, tag="scr")
    nc.vector.tensor_mul(scr[:], pA[:, 0:L], pA[:, L : 2 * L])
    s2_raw = wst.tile([128, 1], F32, tag="s2_raw")
    nc.vector.reduce_sum(s2_raw[:], scr[:], axis=mybir.AxisListType.X)
    affb2p1 = wst.tile([128, 1], F32, tag="affb2p1")
    nc.scalar.activation(
        affb2p1[:], pA[:, 2 * L : 2 * L + 1],
        mybir.ActivationFunctionType.Identity, bias=1.0,
    )
    s2 = wst.tile([128, 1], F32, tag="s2")
    nc.scalar.activation(
        s2[:], s2_raw[:], mybir.ActivationFunctionType.Identity, bias=affb2p1[:]
    )
    s2h = wst.tile([128, 1], F32, tag="s2h")
    nc.vector.tensor_scalar_mul(s2h[:], s2[:], 0.5)
    # qsum[ci, co] = sum_t (s2[ci] * wgt_t[ci, co*9+t])^2
    swT = wst.tile([64, COUT * 9], F32, tag="swT")
    nc.vector.tensor_scalar_mul(swT[:], pB[0:64, 0 : COUT * 9], s2[0:64, :])
    nc.vector.tensor_mul(swT[:], swT[:], swT[:])
    qsum = wst.tile([64, COUT], F32, tag="qsum")
    nc.vector.reduce_sum(
        qsum[:].rearrange("p (c u) -> p c u", u=1),
        swT[:].rearrange("p (c t) -> p c t", t=9),
        axis=mybir.AxisListType.X,
    )
    return {"pB": pB, "s2": s2, "s2h": s2h, "qsum": qsum}


def _prep_late(tc, pools, st, consts):
    """Prep phase B: demod broadcast (tiny PE matmuls, emitted late so the
    in-order PE queue never waits on them) + the 8 winograd stationaries."""
    nc = tc.nc
    wst = pools["wst"]
    ones64, onesrow = consts
    # dsq[co] = sum_ci qsum[ci, co]; d = rsqrt(dsq + eps), bcast to [128,128]
    ps_d = pools["pprep"].tile([1, COUT], F32, tag="pp")
    nc.tensor.matmul(ps_d[:], ones64[:], st["qsum"][:])
    drow = wst.tile([1, 128], F32, tag="drow")
    nc.vector.tensor_scalar_add(drow[:, 0:64], ps_d[:], EPS)
    nc.vector.tensor_scalar_add(drow[:, 64:128], ps_d[:], EPS)
    nc.vector.reciprocal(drow[:], drow[:])
    nc.scalar.activation(drow[:], drow[:], mybir.ActivationFunctionType.Sqrt)
    ps_dbc = pools["pprep"].tile([128, 128], F32, tag="pp")
    nc.tensor.matmul(ps_dbc[:], onesrow[:], drow[:])
    dbc = wst.tile([128, 128], F32, tag="dbc")
    nc.vector.tensor_copy(dbc[:], ps_dbc[:])
    # 8 stationaries: per set (A from kw tiles 0-2, B from 3-5):
    #   ST0=g0, ST1=(g0+g1+g2)/2, ST2=(g0-g1+g2)/2, ST3=g2; x s2 (K), x dbc (M)
    s2, s2h = st["s2"], st["s2h"]
    W0 = COUT * 9
    stat = []
    tsum = wst.tile([128, 128], F32, tag="tsum")
    tmid = wst.tile([128, 128], F32, tag="tmid")
    for base, nm in ((0, "A"), (3, "B")):
        g = [st["pB"][:, W0 + 128 * (base + k) : W0 + 128 * (base + k + 1)]
             for k in range(3)]
        nc.vector.tensor_add(tsum[:], g[0], g[2])
        for p, (src, sc) in enumerate(
            ((g[0], s2), (None, s2h), (None, s2h), (g[2], s2))
        ):
            t = wst.tile([128, 128], F16, tag=f"{nm}{p}")
            if src is None:
                if p == 1:
                    nc.vector.tensor_add(tmid[:], tsum[:], g[1])
                else:
                    nc.vector.tensor_sub(tmid[:], tsum[:], g[1])
                src = tmid[:]
            tscaled = wst.tile([128, 128], F32, tag=f"{nm}{p}s")
            nc.vector.tensor_scalar_mul(tscaled[:], src, sc[:])
            nc.vector.tensor_mul(t[:], tscaled[:], dbc[:])
            stat.append(t)
    st["A"] = stat[0:4]
    st["B"] = stat[4:8]


def _emit_body(tc, pools, xs, out, st, prepA, prepB, consts, do_next, carry):
    """One conv body. `carry` holds the cross-body software-pipeline state:
    pending passB quad + last raw tile (the body halo is body-local, so the
    carry is only within-body; reset each body)."""
    nc = tc.nc
    xsv = xs.rearrange("s ci q c -> (s ci) q c")  # [128, 129, ROWB]
    outv = out.rearrange("s co g u e c -> (s co) (g u e c)")  # [128, 16*4*2*512]

    raws = {}  # raw tile index -> AP
    phases = {}  # phase group -> AP

    def load_raw(i):
        # 9-pair tiles overlapping by one pair: tile i = pairs 8i..8i+8, so a
        # transform group needs exactly one raw tile (no cross-tile halo op)
        if i in raws:
            return
        q0 = 8 * i
        t = pools["xg"].tile([128, (G + 1) * ROWB], F16, tag="xg")
        nc.sync.dma_start(
            t[:],
            xsv[:, q0 : q0 + G + 1, :].rearrange("p q c -> p (q c)"),
        )
        raws[i] = t

    def transform(g):
        """Build phase group g (pairs 8g .. 8g+8) from raw tile g (9 pairs)."""
        ph = pools["ph"].tile([128, 4 * PW], F16, tag="ph")
        big = raws[g][:].rearrange("p (b c) -> p b c", c=ROWB)
        # slices of a row: Xe[j]=0:256, Xe[j+1]=1:257, Xo[j]=257:513, Xo[j+1]=258:514
        ops = [
            (0, 0, 1, "sub"),      # V0 = Xe[j] - Xe[j+1]
            (1, 257, 1, "add"),    # V1 = Xo[j] + Xe[j+1]
            (2, 1, 257, "sub"),    # V2 = Xe[j+1] - Xo[j]
            (3, 257, 258, "sub"),  # V3 = Xo[j] - Xo[j+1]
        ]
        for p, a0, b0, op in ops:
            # gpsimd has ~2us fixed overhead per op: give it exactly one big
            # phase op per group; the rest go to DVE
            eng = nc.gpsimd if p == 3 else nc.vector
            f = eng.tensor_sub if op == "sub" else eng.tensor_add
            f(
                ph[:, p * PW : p * PW + PW].rearrange("p (b c) -> p b c", c=NJ),
                big[:, :, a0 : a0 + NJ],
                big[:, :, b0 : b0 + NJ],
            )
        phases[g] = ph

    def pass_a(u):
        """First 4 matmuls of quad u (m1 into bank1, m2 into bank2) + the
        mid-group ACT drains of m1 (f32, for og_odd on DVE) and m2 (f16,
        for og_even on GpSimd)."""
        g, qu = divmod(u, 4)
        ph = phases[g][:]
        ps = pools["ps"].tile([128, 1024], F32, tag="ps")
        m1sb = pools["msb"].tile([128, 512], F32, tag="m1sb")
        m2sb = pools["msb"].tile([128, 512], F16, tag="m2sb")
        qq = 2 * (qu % 4) * NJ  # rel col of pair 2u within the phase group
        for bank, p in ((0, 1), (1, 2)):
            bv = ps[:, bank * 512 : bank * 512 + 512]
            nc.tensor.matmul(bv, st["A"][p][:],
                             ph[:, p * PW + qq : p * PW + qq + 512],
                             start=True, stop=False)
            nc.tensor.matmul(bv, st["B"][p][:],
                             ph[:, p * PW + qq + NJ : p * PW + qq + NJ + 512],
                             start=False, stop=False)
        nc.scalar.activation(m1sb[:], ps[:, 0:512],
                             mybir.ActivationFunctionType.Copy)
        nc.scalar.activation(m2sb[:], ps[:, 512:1024],
                             mybir.ActivationFunctionType.Copy)
        return {"ps": ps, "m1sb": m1sb, "m2sb": m2sb, "u": u}

    def pass_b(qa, og):
        """Last 4 matmuls of quad u (+m0 into bank1, +m3 into bank2), ACT
        drain of bank1 (m0+m1, f16), then the inverse combines: og_even on
        GpSimd (all-f16 SBUF), og_odd on DVE (psum operand)."""
        u = qa["u"]
        g, qu = divmod(u, 4)
        ph = phases[g][:]
        ps = qa["ps"]
        qq = 2 * (qu % 4) * NJ
        for bank, p in ((0, 0), (1, 3)):
            bv = ps[:, bank * 512 : bank * 512 + 512]
            nc.tensor.matmul(bv, st["A"][p][:],
                             ph[:, p * PW + qq : p * PW + qq + 512],
                             start=False, stop=False)
            nc.tensor.matmul(bv, st["B"][p][:],
                             ph[:, p * PW + qq + NJ : p * PW + qq + NJ + 512],
                             start=False, stop=True)
        b1sb = pools["msb"].tile([128, 512], F16, tag="b1sb")
        nc.scalar.activation(b1sb[:], ps[:, 0:512],
                             mybir.ActivationFunctionType.Copy)
        o = og[:, qu * 1024 : qu * 1024 + 1024]
        nc.vector.tensor_add(o[:, 0:512], b1sb[:], qa["m2sb"][:])
        nc.vector.tensor_sub(o[:, 512:1024], qa["m1sb"][:], ps[:, 512:1024])

    st_next = None
    load_raw(0)
    transform(0)
    load_raw(1)
    pend = None  # pass-A state of the previous quad (1-quad software lag)
    ogs = {}
    for g in range(NOG):
        load_raw(min(g + 2, NOG - 1))
        if g + 1 < NOG:
            transform(g + 1)
        if do_next and g == PREP_EARLY:
            st_next = _prep_early(tc, pools, prepA, prepB)
        if do_next and g == PREP_LATE:
            _prep_late(tc, pools, st_next, consts)
        ogs[g] = pools["og"].tile([128, 4 * 1024], F16, tag="og", name="og")
        for qu in range(4):
            u = 4 * g + qu
            qa = pass_a(u)
            if pend is not None:
                up = pend["u"]
                pass_b(pend, ogs[up // 4])
                if up % 4 == 3:
                    gp = up // 4
                    nc.sync.dma_start(
                        outv[:, gp * 4096 : (gp + 1) * 4096], ogs[gp][:]
                    )
                    del ogs[gp]
            pend = qa
    pass_b(pend, ogs[NOG - 1])
    nc.sync.dma_start(outv[:, (NOG - 1) * 4096 : NOG * 4096], ogs[NOG - 1][:])
    return st_next if st_next is not None else st


def _emit_all(tc, xs, prepA, prepB, out, reps):
    nc = tc.nc
    pools = {
        "const": tc.alloc_tile_pool(name="const", bufs=1),
        "wst": tc.alloc_tile_pool(name="wst", bufs=2),
        "pprep": tc.alloc_tile_pool(name="pprep", bufs=1, space="PSUM"),
        "xg": tc.alloc_tile_pool(name="xg", bufs=XBUFS),
        "ph": tc.alloc_tile_pool(name="ph", bufs=PHBUFS),
        "og": tc.alloc_tile_pool(name="og", bufs=3),
        "msb": tc.alloc_tile_pool(name="msb", bufs=3),
        "ps": tc.alloc_tile_pool(name="ps", bufs=3, space="PSUM"),
    }
    ones64 = pools["const"].tile([64, 1], F32, tag="ones64")
    nc.vector.memset(ones64[:], 1.0)
    onesrow = pools["const"].tile([1, 128], F32, tag="onesrow")
    nc.vector.memset(onesrow[:], 1.0)
    consts = (ones64, onesrow)

    st = _prep_early(tc, pools, prepA, prepB)
    _prep_late(tc, pools, st, consts)
    for r in range(reps):
        st = _emit_body(tc, pools, xs, out, st, prepA, prepB, consts,
                        do_next=(r < reps - 1), carry=None)
    for p in ["ps", "msb", "og", "ph", "xg", "pprep", "wst", "const"]:
        pools[p].release()


def _get_nc(reps=1, hw_iters=1):
    key = (reps, hw_iters)
    if key not in _CACHE:
        _CACHE[key] = _build_nc(reps, hw_iters)
    return _CACHE[key]


def _host_weight_layouts(weight):
    """prepB: wgt_t [ci, co*9] + six raw kw stationary tiles, packed."""
    lhsT6 = np.zeros((6, 128, 128), dtype=np.float32)
    wt = np.ascontiguousarray(weight.transpose(1, 0, 2, 3))  # [ci, co, kh, kw]
    for kw in range(3):
        a, b = lhsT6[kw], lhsT6[3 + kw]
        a[0:64, 0:64] = wt[:, :, 0, kw]
        a[64:128, 0:64] = wt[:, :, 1, kw]
        a[64:128, 64:128] = wt[:, :, 0, kw]
        b[0:64, 0:64] = wt[:, :, 2, kw]
        b[0:64, 64:128] = wt[:, :, 1, kw]
        b[64:128, 64:128] = wt[:, :, 2, kw]
    prepB = np.zeros((128, PB_COLS), dtype=np.float32)
    prepB[0:64, 0 : COUT * 9] = wt.reshape(CIN, COUT * 9)
    prepB[:, COUT * 9 :] = lhsT6.transpose(1, 0, 2).reshape(128, 6 * 128)
    return prepB


def _shard_inputs(x, w, weight, affine_w, affine_b):
    """Build the 8 per-core input maps (host-side slicing + halo padding +
    even/odd column split)."""
    prepB = _host_weight_layouts(np.asarray(weight, dtype=np.float32))
    affw = np.asarray(affine_w, dtype=np.float32)
    affb = np.asarray(affine_b, dtype=np.float32)
    w = np.asarray(w, dtype=np.float32)
    x = np.asarray(x, dtype=np.float32)
    prepA_base = np.zeros((128, PA_COLS), dtype=np.float32)
    prepA_base[0:64, 0:L] = affw
    prepA_base[64:128, 0:L] = affw
    prepA_base[0:64, 2 * L] = affb
    prepA_base[64:128, 2 * L] = affb
    in_maps = []
    for core in range(N_CORES):
        b, half = divmod(core, 2)
        h0 = half * HALF
        xsh = np.zeros((CIN, R_IN, ROWB), dtype=np.float32)
        lo, hi = h0 - 1, h0 + HALF + 1  # global rows [lo, hi)
        clo, chi = max(lo, 0), min(hi, H)
        xsh[:, clo - lo : chi - lo, 1 : 1 + W] = x[b, :, clo:chi, :]
        # even/odd column split per row: [Xe(257) | Xo(257)]
        xeo = np.concatenate([xsh[:, :, 0::2], xsh[:, :, 1::2]], axis=2)
        # parity-split layout: xs2[s, ci, q, :] = padded row 2q+s
        xs2 = np.ascontiguousarray(
            xeo.reshape(CIN, R_IN // 2, 2, ROWB).transpose(2, 0, 1, 3)
        ).astype(np.float16)
        prepA = prepA_base.copy()
        prepA[:, L : 2 * L] = w[b][None, :]
        in_maps.append({"xs": xs2, "prepA": prepA, "prepB": prepB})
    return in_maps


def _decode_core(o2):
    """[2,COUT,16,4,2,512] device layout -> [COUT, HALF, W] rows/cols."""
    # dims [s, co, U=(g,u), e, pp, j] -> row 4U+2pp+s, col 2j+e
    o6 = np.asarray(o2, dtype=np.float32).reshape(2, COUT, NQUAD, 2, 2, NJ)
    return o6.transpose(1, 2, 4, 0, 5, 3).reshape(COUT, HALF, W)


def kernel(x, w, weight, affine_w, affine_b):
    nc = _get_nc()
    in_maps = _shard_inputs(x, w, weight, affine_w, affine_b)
    res = run_bass_kernel_spmd(nc, in_maps, list(range(N_CORES)))
    full = np.empty((B, COUT, H, W), dtype=np.float32)
    for core in range(N_CORES):
        b, half = divmod(core, 2)
        full[b, :, half * HALF : (half + 1) * HALF, :] = _decode_core(
            res.results[core]["out"]
        )
    return full


# revision 20
# speedup vs baseline: 1.3165x; 1.2379x over previous
"""Modulated 3x3 conv (StyleGAN2-style, groups=B) on 8 trn2 NeuronCores.

Sharding: data-parallel over (batch B=4) x (image half H/2), 8 shards.
Each core computes a full 64->64 channel 3x3 conv over a 256-row half of
one sample's 512x512 image, via Winograd F(2,3) along the W axis.

Why Winograd: the direct 6-matmul/row-pair schedule is PE-streaming-bound
at 768 N=512 matmuls/body, and the sustained HAM power throttle caps the
tensor engine at k=13/16 duty, so the only lever is fewer PE cycles.
F(2,3) along W replaces the 3 kw-shifted matmuls with 4 phase matmuls at
HALF the free dim, cutting streamed columns 1.5x: 512 N=512 matmuls/body.

Scheme per "quad" (2 output row pairs = 4 output rows):
  phases p=0..3 of the input rows (V0=Xe[j]-Xe[j+1], V1=Xo[j]+Xe[j+1],
  V2=Xe[j+1]-Xo[j], V3=Xo[j]-Xo[j+1], from a host-prepared even/odd
  column split), and kw-transformed stationaries A_p/B_p (g'=G g along kw,
  kh packed in quadrants as in the direct kernel; style s on K, demod d on
  M). PSUM tile = 2 banks: bank1 accumulates m1 then +m0, bank2 m2 then
  +m3 (4 matmuls each, N=512 covering both row pairs). Mid-accumulation
  ACT drains m1,m2 to SBUF; the inverse transform is then just two DVE
  combines: out_even = bank1 + m2_sb, out_odd = m1_sb - bank2.

All pools persist across the unrolled bodies; the next body's weight prep
is double-buffered and emitted inside the current body (DMA+vector chain
early, PE reduce matmuls + stationary builds late) so the PE stream never
idles at body boundaries (idle would also re-trigger HAM). Input
transforms run on DVE+GpSimd a group ahead of the matmuls; output stores
issue from the ACT HWDGE queue so the Sync queue carries only x loads.
The host only slices/pads/casts/relayouts (even/odd split, output
interleave decode); s and d are computed on-device per body.
"""

import numpy as np

import concourse.bacc as bacc
import concourse.mybir as mybir
import concourse.tile as tile
from concourse.bass_utils import run_bass_kernel_spmd

B, CIN, COUT, L, H, W = 4, 64, 64, 512, 512, 512
N_CORES = 8
HALF = H // 2  # 256 output rows per core
R_IN = HALF + 2  # 258 input rows per core (1 halo/pad row each side)
NPAIR = HALF // 2  # 128 output row pairs per core
NQUAD = NPAIR // 2  # 64
G = 8  # row pairs per group
NOG = NPAIR // G  # 16 groups
ROWB = W + 2  # per-row SBUF cols: [Xe 257 | Xo 257]
EO = ROWB // 2  # 257
NJ = W // 2  # 256 output col-pairs per row
EPS = 1e-8
F32 = mybir.dt.float32
F16 = mybir.dt.float16
XBUFS = 4  # raw x tile ring
PHBUFS = 3  # phase tile ring
PW = (G + 1) * NJ  # phase block stride (9 pairs x 256)
PREP_EARLY = 2
PREP_LATE = 12

PA_COLS = 2 * L + 1  # 1025: affw dup | w bcast | affb dup
PB_COLS = COUT * 9 + 6 * 128  # 1344: wgt_t | six raw stationaries

_CACHE = {}


def _build_nc(reps=1, hw_iters=1):
    nc = bacc.Bacc("TRN2", target_bir_lowering=False, debug=False)
    # xs[s, ci, q, :]: padded row 2q+s, cols = [Xe(257) | Xo(257)]
    xs = nc.dram_tensor("xs", [2, CIN, R_IN // 2, ROWB], F16, kind="ExternalInput")
    prepA = nc.dram_tensor("prepA", [128, PA_COLS], F32, kind="ExternalInput")
    prepB = nc.dram_tensor("prepB", [128, PB_COLS], F32, kind="ExternalInput")
    # out[s, co, group, quad, eo, c]: c = pair-in-quad*256 + j; col = 2j+eo
    out = nc.dram_tensor("out", [2, COUT, NOG, 4, 2, W], F16, kind="ExternalOutput")

    with tile.TileContext(nc) as tc:
        if hw_iters > 1:
            with tc.For_i(0, hw_iters, 1):
                _emit_all(tc, xs, prepA, prepB, out, reps)
        else:
            _emit_all(tc, xs, prepA, prepB, out, reps)
    nc.compile()
    return nc


def _prep_early(tc, pools, prepA, prepB):
    """Prep phase A: packed DMAs + style/demod vector chains (Sync/DVE/ACT)."""
    nc = tc.nc
    wst = pools["wst"]
    pA = wst.tile([128, PA_COLS], F32, tag="pA")
    pB = wst.tile([128, PB_COLS], F32, tag="pB")
    nc.sync.dma_start(pA[:], prepA[:, :])
    nc.sync.dma_start(pB[:], prepB[:, :])
    # s2[p] = sum_l affw[p%64, l] * w[l] + affb[p%64] + 1  (gpsimd: SBUF-only
    # prep work stays off the saturated DVE)
    scr = wst.tile([128, L], F32, tag="scr")
    nc.vector.tensor_mul(scr[:], pA[:, 0:L], pA[:, L : 2 * L])
    s2_raw = wst.tile([128, 1], F32, tag="s2_raw")
    nc.vector.reduce_sum(s2_raw[:], scr[:], axis=mybir.AxisListType.X)
    affb2p1 = wst.tile([128, 1], F32, tag="affb2p1")
    nc.scalar.activation(
        affb2p1[:], pA[:, 2 * L : 2 * L + 1],
        mybir.ActivationFunctionType.Identity, bias=1.0,
    )
    s2 = wst.tile([128, 1], F32, tag="s2")
    nc.scalar.activation(
        s2[:], s2_raw[:], mybir.ActivationFunctionType.Identity, bias=affb2p1[:]
    )
    s2h = wst.tile([128, 1], F32, tag="s2h")
    nc.vector.tensor_scalar_mul(s2h[:], s2[:], 0.5)
    # qsum[ci, co] = sum_t (s2[ci] * wgt_t[ci, co*9+t])^2
    swT = wst.tile([64, COUT * 9], F32, tag="swT")
    nc.vector.tensor_scalar_mul(swT[:], pB[0:64, 0 : COUT * 9], s2[0:64, :])
    nc.vector.tensor_mul(swT[:], swT[:], swT[:])
    qsum = wst.tile([64, COUT], F32, tag="qsum")
    nc.vector.reduce_sum(
        qsum[:].rearrange("p (c u) -> p c u", u=1),
        swT[:].rearrange("p (c t) -> p c t", t=9),
        axis=mybir.AxisListType.X,
    )
    return {"pB": pB, "s2": s2, "s2h": s2h, "qsum": qsum}


def _prep_late(tc, pools, st, consts):
    """Prep phase B: demod broadcast (tiny PE matmuls, emitted late so the
    in-order PE queue never waits on them) + the 8 winograd stationaries."""
    nc = tc.nc
    wst = pools["wst"]
    ones64, onesrow = consts
    # dsq[co] = sum_ci qsum[ci, co]; d = rsqrt(dsq + eps), bcast to [128,128]
    ps_d = pools["pprep"].tile([1, COUT], F32, tag="pp")
    nc.tensor.matmul(ps_d[:], ones64[:], st["qsum"][:])
    drow = wst.tile([1, 128], F32, tag="drow")
    nc.vector.tensor_scalar_add(drow[:, 0:64], ps_d[:], EPS)
    nc.vector.tensor_scalar_add(drow[:, 64:128], ps_d[:], EPS)
    nc.vector.reciprocal(drow[:], drow[:])
    nc.scalar.activation(drow[:], drow[:], mybir.ActivationFunctionType.Sqrt)
    ps_dbc = pools["pprep"].tile([128, 128], F32, tag="pp")
    nc.tensor.matmul(ps_dbc[:], onesrow[:], drow[:])
    dbc = wst.tile([128, 128], F32, tag="dbc")
    nc.vector.tensor_copy(dbc[:], ps_dbc[:])
    # 8 stationaries: per set (A from kw tiles 0-2, B from 3-5):
    #   ST0=g0, ST1=(g0+g1+g2)/2, ST2=(g0-g1+g2)/2, ST3=g2; x s2 (K), x dbc (M)
    s2, s2h = st["s2"], st["s2h"]
    W0 = COUT * 9
    stat = []
    tsum = wst.tile([128, 128], F32, tag="tsum")
    tmid = wst.tile([128, 128], F32, tag="tmid")
    for base, nm in ((0, "A"), (3, "B")):
        g = [st["pB"][:, W0 + 128 * (base + k) : W0 + 128 * (base + k + 1)]
             for k in range(3)]
        nc.vector.tensor_add(tsum[:], g[0], g[2])
        for p, (src, sc) in enumerate(
            ((g[0], s2), (None, s2h), (None, s2h), (g[2], s2))
        ):
            t = wst.tile([128, 128], F16, tag=f"{nm}{p}")
            if src is None:
                if p == 1:
                    nc.vector.tensor_add(tmid[:], tsum[:], g[1])
                else:
                    nc.vector.tensor_sub(tmid[:], tsum[:], g[1])
                src = tmid[:]
            tscaled = wst.tile([128, 128], F32, tag=f"{nm}{p}s")
            nc.vector.tensor_scalar_mul(tscaled[:], src, sc[:])
            nc.vector.tensor_mul(t[:], tscaled[:], dbc[:])
            stat.append(t)
    st["A"] = stat[0:4]
    st["B"] = stat[4:8]


def _emit_body(tc, pools, xs, out, st, prepA, prepB, consts, do_next, carry):
    """One conv body. `carry` holds the cross-body software-pipeline state:
    pending passB quad + last raw tile (the body halo is body-local, so the
    carry is only within-body; reset each body)."""
    nc = tc.nc
    xsv = xs.rearrange("s ci q c -> (s ci) q c")  # [128, 129, ROWB]
    outv = out.rearrange("s co g u e c -> (s co) (g u e c)")  # [128, 16*4*2*512]

    raws = {}  # raw tile index -> AP
    phases = {}  # phase group -> AP

    def load_raw(i):
        # 9-pair tiles overlapping by one pair: tile i = pairs 8i..8i+8, so a
        # transform group needs exactly one raw tile (no cross-tile halo op)
        if i in raws:
            return
        q0 = 8 * i
        t = pools["xg"].tile([128, (G + 1) * ROWB], F16, tag="xg")
        nc.sync.dma_start(
            t[:],
            xsv[:, q0 : q0 + G + 1, :].rearrange("p q c -> p (q c)"),
        )
        raws[i] = t

    def transform(g):
        """Build phase group g (pairs 8g .. 8g+8) from raw tile g (9 pairs)."""
        ph = pools["ph"].tile([128, 4 * PW], F16, tag="ph")
        big = raws[g][:].rearrange("p (b c) -> p b c", c=ROWB)
        # slices of a row: Xe[j]=0:256, Xe[j+1]=1:257, Xo[j]=257:513, Xo[j+1]=258:514
        ops = [
            (0, 0, 1, "sub"),      # V0 = Xe[j] - Xe[j+1]
            (1, 257, 1, "add"),    # V1 = Xo[j] + Xe[j+1]
            (2, 1, 257, "sub"),    # V2 = Xe[j+1] - Xo[j]
            (3, 257, 258, "sub"),  # V3 = Xo[j] - Xo[j+1]
        ]
        for p, a0, b0, op in ops:
            # all DVE: gpsimd pays ~2us fixed overhead per op and cross-
            # engine chains through it have measured worse than DVE-saturated
            f = nc.vector.tensor_sub if op == "sub" else nc.vector.tensor_add
            f(
                ph[:, p * PW : p * PW + PW].rearrange("p (b c) -> p b c", c=NJ),
                big[:, :, a0 : a0 + NJ],
                big[:, :, b0 : b0 + NJ],
            )
        phases[g] = ph

    def pass_a(u):
        """First 4 matmuls of quad u (m1 into bank1, m2 into bank2) + the
        mid-group ACT drains of m1 (f32, for og_odd on DVE) and m2 (f16,
        for og_even on GpSimd)."""
        g, qu = divmod(u, 4)
        ph = phases[g][:]
        ps = pools["ps"].tile([128, 1024], F32, tag="ps")
        m1sb = pools["msb"].tile([128, 512], F32, tag="m1sb")
        m2sb = pools["msb"].tile([128, 512], F32, tag="m2sb")
        qq = 2 * (qu % 4) * NJ  # rel col of pair 2u within the phase group
        for bank, p in ((0, 1), (1, 2)):
            bv = ps[:, bank * 512 : bank * 512 + 512]
            nc.tensor.matmul(bv, st["A"][p][:],
                             ph[:, p * PW + qq : p * PW + qq + 512],
                             start=True, stop=False)
            nc.tensor.matmul(bv, st["B"][p][:],
                             ph[:, p * PW + qq + NJ : p * PW + qq + NJ + 512],
                             start=False, stop=False)
        nc.scalar.activation(m1sb[:], ps[:, 0:512],
                             mybir.ActivationFunctionType.Copy)
        nc.scalar.activation(m2sb[:], ps[:, 512:1024],
                             mybir.ActivationFunctionType.Copy)
        return {"ps": ps, "m1sb": m1sb, "m2sb": m2sb, "u": u}

    def pass_b(qa, og):
        """Last 4 matmuls of quad u (+m0 into bank1, +m3 into bank2), ACT
        drain of bank1 (m0+m1, f16), then the inverse combines: og_even on
        GpSimd (all-f16 SBUF), og_odd on DVE (psum operand)."""
        u = qa["u"]
        g, qu = divmod(u, 4)
        ph = phases[g][:]
        ps = qa["ps"]
        qq = 2 * (qu % 4) * NJ
        for bank, p in ((0, 0), (1, 3)):
            bv = ps[:, bank * 512 : bank * 512 + 512]
            nc.tensor.matmul(bv, st["A"][p][:],
                             ph[:, p * PW + qq : p * PW + qq + 512],
                             start=False, stop=False)
            nc.tensor.matmul(bv, st["B"][p][:],
                             ph[:, p * PW + qq + NJ : p * PW + qq + NJ + 512],
                             start=False, stop=True)
        o = og[:, qu * 1024 : qu * 1024 + 1024]
        nc.vector.tensor_add(o[:, 0:512], ps[:, 0:512], qa["m2sb"][:])
        nc.vector.tensor_sub(o[:, 512:1024], qa["m1sb"][:], ps[:, 512:1024])

    st_next = None
    load_raw(0)
    transform(0)
    load_raw(1)
    pend = None  # pass-A state of the previous quad (1-quad software lag)
    ogs = {}
    for g in range(NOG):
        load_raw(min(g + 2, NOG - 1))
        if g + 1 < NOG:
            transform(g + 1)
        if do_next and g == PREP_EARLY:
            st_next = _prep_early(tc, pools, prepA, prepB)
        if do_next and g == PREP_LATE:
            _prep_late(tc, pools, st_next, consts)
        ogs[g] = pools["og"].tile([128, 4 * 1024], F16, tag="og", name="og")
        for qu in range(4):
            u = 4 * g + qu
            qa = pass_a(u)
            if pend is not None:
                up = pend["u"]
                pass_b(pend, ogs[up // 4])
                if up % 4 == 3:
                    gp = up // 4
                    nc.sync.dma_start(
                        outv[:, gp * 4096 : (gp + 1) * 4096], ogs[gp][:]
                    )
                    del ogs[gp]
            pend = qa
    pass_b(pend, ogs[NOG - 1])
    nc.sync.dma_start(outv[:, (NOG - 1) * 4096 : NOG * 4096], ogs[NOG - 1][:])
    return st_next if st_next is not None else st


def _emit_all(tc, xs, prepA, prepB, out, reps):
    nc = tc.nc
    pools = {
        "const": tc.alloc_tile_pool(name="const", bufs=1),
        "wst": tc.alloc_tile_pool(name="wst", bufs=2),
        "pprep": tc.alloc_tile_pool(name="pprep", bufs=1, space="PSUM"),
        "xg": tc.alloc_tile_pool(name="xg", bufs=XBUFS),
        "ph": tc.alloc_tile_pool(name="ph", bufs=PHBUFS),
        "og": tc.alloc_tile_pool(name="og", bufs=3),
        "msb": tc.alloc_tile_pool(name="msb", bufs=3),
        "ps": tc.alloc_tile_pool(name="ps", bufs=3, space="PSUM"),
    }
    ones64 = pools["const"].tile([64, 1], F32, tag="ones64")
    nc.vector.memset(ones64[:], 1.0)
    onesrow = pools["const"].tile([1, 128], F32, tag="onesrow")
    nc.vector.memset(onesrow[:], 1.0)
    consts = (ones64, onesrow)

    st = _prep_early(tc, pools, prepA, prepB)
    _prep_late(tc, pools, st, consts)
    for r in range(reps):
        st = _emit_body(tc, pools, xs, out, st, prepA, prepB, consts,
                        do_next=(r < reps - 1), carry=None)
    for p in ["ps", "msb", "og", "ph", "xg", "pprep", "wst", "const"]:
        pools[p].release()


def _get_nc(reps=1, hw_iters=1):
    key = (reps, hw_iters)
    if key not in _CACHE:
        _CACHE[key] = _build_nc(reps, hw_iters)
    return _CACHE[key]


def _host_weight_layouts(weight):
    """prepB: wgt_t [ci, co*9] + six raw kw stationary tiles, packed."""
    lhsT6 = np.zeros((6, 128, 128), dtype=np.float32)
    wt = np.ascontiguousarray(weight.transpose(1, 0, 2, 3))  # [ci, co, kh, kw]
    for kw in range(3):
        a, b = lhsT6[kw], lhsT6[3 + kw]
        a[0:64, 0:64] = wt[:, :, 0, kw]
        a[64:128, 0:64] = wt[:, :, 1, kw]
        a[64:128, 64:128] = wt[:, :, 0, kw]
        b[0:64, 0:64] = wt[:, :, 2, kw]
        b[0:64, 64:128] = wt[:, :, 1, kw]
        b[64:128, 64:128] = wt[:, :, 2, kw]
    prepB = np.zeros((128, PB_COLS), dtype=np.float32)
    prepB[0:64, 0 : COUT * 9] = wt.reshape(CIN, COUT * 9)
    prepB[:, COUT * 9 :] = lhsT6.transpose(1, 0, 2).reshape(128, 6 * 128)
    return prepB


def _shard_inputs(x, w, weight, affine_w, affine_b):
    """Build the 8 per-core input maps (host-side slicing + halo padding +
    even/odd column split)."""
    prepB = _host_weight_layouts(np.asarray(weight, dtype=np.float32))
    affw = np.asarray(affine_w, dtype=np.float32)
    affb = np.asarray(affine_b, dtype=np.float32)
    w = np.asarray(w, dtype=np.float32)
    x = np.asarray(x, dtype=np.float32)
    prepA_base = np.zeros((128, PA_COLS), dtype=np.float32)
    prepA_base[0:64, 0:L] = affw
    prepA_base[64:128, 0:L] = affw
    prepA_base[0:64, 2 * L] = affb
    prepA_base[64:128, 2 * L] = affb
    in_maps = []
    for core in range(N_CORES):
        b, half = divmod(core, 2)
        h0 = half * HALF
        xsh = np.zeros((CIN, R_IN, ROWB), dtype=np.float32)
        lo, hi = h0 - 1, h0 + HALF + 1  # global rows [lo, hi)
        clo, chi = max(lo, 0), min(hi, H)
        xsh[:, clo - lo : chi - lo, 1 : 1 + W] = x[b, :, clo:chi, :]
        # even/odd column split per row: [Xe(257) | Xo(257)]
        xeo = np.concatenate([xsh[:, :, 0::2], xsh[:, :, 1::2]], axis=2)
        # parity-split layout: xs2[s, ci, q, :] = padded row 2q+s
        xs2 = np.ascontiguousarray(
            xeo.reshape(CIN, R_IN // 2, 2, ROWB).transpose(2, 0, 1, 3)
        ).astype(np.float16)
        prepA = prepA_base.copy()
        prepA[:, L : 2 * L] = w[b][None, :]
        in_maps.append({"xs": xs2, "prepA": prepA, "prepB": prepB})
    return in_maps


def _decode_core(o2):
    """[2,COUT,16,4,2,512] device layout -> [COUT, HALF, W] rows/cols."""
    # dims [s, co, U=(g,u), e, pp, j] -> row 4U+2pp+s, col 2j+e
    o6 = np.asarray(o2, dtype=np.float32).reshape(2, COUT, NQUAD, 2, 2, NJ)
    return o6.transpose(1, 2, 4, 0, 5, 3).reshape(COUT, HALF, W)


def kernel(x, w, weight, affine_w, affine_b):
    nc = _get_nc()
    in_maps = _shard_inputs(x, w, weight, affine_w, affine_b)
    res = run_bass_kernel_spmd(nc, in_maps, list(range(N_CORES)))
    full = np.empty((B, COUT, H, W), dtype=np.float32)
    for core in range(N_CORES):
        b, half = divmod(core, 2)
        full[b, :, half * HALF : (half + 1) * HALF, :] = _decode_core(
            res.results[core]["out"]
        )
    return full
